# revision 1
# baseline (speedup 1.0000x reference)
"""AttnBlock (GroupNorm + 1x1-conv QKV + NxN attention + proj + residual) on 8 NeuronCores.

Sharding: data-parallel over batch (4 samples) x 2-way sequence-parallel over
query rows. Each core gets one sample's full (C,N) activation with its query
half permuted to columns 0:2048, computes GroupNorm stats, normalizes, runs
scores/softmax/AV in a j-transposed layout (so no on-chip transposes are
needed anywhere), and emits its 2048 output columns transposed (positions on
partitions) so the softmax denominator can be applied as a per-partition
scalar.

All heavy matmuls run in bf16 with fp32 PSUM accumulation; statistics,
softmax denominators and the residual path stay in fp32.
"""

import numpy as np
import ml_dtypes
from contextlib import ExitStack

import concourse.bass as bass
import concourse.bacc as bacc
import concourse.mybir as mybir
import concourse.tile as tile
from concourse.tile_rust import add_dep_helper
from concourse.bass_utils import run_bass_kernel_spmd

F32 = mybir.dt.float32
BF16 = mybir.dt.bfloat16
AF = mybir.ActivationFunctionType
ALU = mybir.AluOpType

C = 512          # channels
NSEQ = 4096      # sequence length (H*W)
NQ = 2048        # query rows per core (sequence-parallel 2-way)
P = 128          # partitions
NCH = C // P     # 4 channel chunks
NJ = NSEQ // P   # 32 key-position chunks
NI = NQ // 512   # 4 query chunks of 512
EPS = 1e-6
SCALE = float(C) ** -0.5
CNT_INV = 1.0 / (16 * NSEQ)   # elements per group (16 ch x 4096 positions)


def build_nc(with_vbias=True):
    nc = bacc.Bacc("TRN2", target_bir_lowering=False, debug=False)

    x_d = nc.dram_tensor("x", [C, NSEQ], BF16, kind="ExternalInput")
    wqT_d = nc.dram_tensor("wqT", [C, C], BF16, kind="ExternalInput")
    wkT_d = nc.dram_tensor("wkT", [C, C], BF16, kind="ExternalInput")
    wvT_d = nc.dram_tensor("wvT", [C, C], BF16, kind="ExternalInput")
    wpT_d = nc.dram_tensor("wpT", [C, C], BF16, kind="ExternalInput")
    # packed per-channel vectors: cols 0=bq 1=bk 2=gn_w 3=gn_b
    bpk_d = nc.dram_tensor("bpk", [C, 4], F32, kind="ExternalInput")
    bvr_d = nc.dram_tensor("bvr", [1, C], BF16, kind="ExternalInput")
    g_d = nc.dram_tensor("gmat", [P, P], F32, kind="ExternalInput")
    xpbT_d = nc.dram_tensor("xpbT", [NQ, C], F32, kind="ExternalInput")
    out_d = nc.dram_tensor("outT", [NQ, C], F32, kind="ExternalOutput")

    x_3d = x_d.rearrange("(c p) n -> p c n", p=P)
    bpk_3d = bpk_d.rearrange("(c p) k -> p c k", p=P)

    with tile.TileContext(nc) as tc, ExitStack() as ctx:
        psum = ctx.enter_context(tc.tile_pool(name="psum", bufs=4, space="PSUM"))
        consts = ctx.enter_context(tc.tile_pool(name="consts", bufs=1))
        wpool = ctx.enter_context(tc.tile_pool(name="wpool", bufs=1))
        hp = ctx.enter_context(tc.tile_pool(name="hp", bufs=1))
        h_sb = []
        for ci in range(NCH):
            t = hp.tile([P, NSEQ], BF16, tag=f"h{ci}", name=f"h{ci}")
            h_sb.append(t)

        # ---- x loads first: one DMA per channel chunk so the stats can
        # pipeline behind the transfers ----
        xsp_cm = tc.tile_pool(name="xsp", bufs=1)
        xsp = xsp_cm.__enter__()
        xs_t = []
        HS = NSEQ // 2
        for ci in range(NCH):
            xs = xsp.tile([P, NSEQ], BF16, tag=f"xs{ci}", bufs=1,
                          name=f"xs{ci}")
            for hf in range(2):
                nc.sync.dma_start(xs[:, hf * HS:(hf + 1) * HS],
                                  x_3d[:, ci, hf * HS:(hf + 1) * HS])
            xs_t.append(xs)

        # constants (issued after x so their queue slots don't delay it)
        g_sb = consts.tile([P, P], F32, tag="g")
        nc.sync.dma_start(g_sb[:], g_d[:])
        bpk_all = consts.tile([P, NCH, 4], F32, tag="bpk")
        nc.sync.dma_start(bpk_all[:], bpk_3d)
        bpk_sb = [bpk_all[:, ci, :] for ci in range(NCH)]
        bvr_sb = consts.tile([1, C], BF16, tag="bvr")
        nc.sync.dma_start(bvr_sb[:], bvr_d[:])
        ones_row = consts.tile([1, P], BF16, tag="ones1")
        nc.vector.memset(ones_row[:], 1.0)
        ones_col = consts.tile([P, 1], F32, tag="ones2")
        nc.vector.memset(ones_col[:], 1.0)

        wt = {}
        for wn, wd in (("q", wqT_d), ("k", wkT_d), ("v", wvT_d), ("p", wpT_d)):
            wall = wpool.tile([P, NCH, C], BF16, tag=f"w{wn}", name=f"w{wn}")
            nc.sync.dma_start(wall[:], wd.rearrange("(c p) n -> p c n", p=P))
            wt[wn] = [wall[:, ci, :] for ci in range(NCH)]

        # ---- per-chunk stats ----
        if True:
            sp_t, ssp_t = [], []
            for ci in range(NCH):
                xs = xs_t[ci]
                st = consts.tile([P, 2], F32, tag=f"s{ci}", name=f"s{ci}")
                sst = consts.tile([P, 2], F32, tag=f"ss{ci}", name=f"ss{ci}")
                for hf in range(2):
                    hsl = slice(hf * HS, (hf + 1) * HS)
                    nc.vector.tensor_reduce(st[:, hf:hf + 1], xs[:, hsl],
                                            axis=mybir.AxisListType.X,
                                            op=ALU.add)
                    sq = xsp.tile([P, HS], BF16, tag="sq", bufs=2,
                                  name=f"sq{ci}_{hf}")
                    nc.scalar.activation(sq[:], xs[:, hsl], AF.Square,
                                         accum_out=sst[:, hf:hf + 1])
                sp_t.append(st)
                ssp_t.append(sst)

            # ---- group stats -> per-channel affine A, B (batched; st8
            # columns are (s0,ss0,s1,ss1,...)) ----
            st8 = consts.tile([P, 2 * NCH], F32, tag="st8")
            for ci in range(NCH):
                nc.vector.tensor_reduce(st8[:, 2 * ci:2 * ci + 1], sp_t[ci][:],
                                        axis=mybir.AxisListType.X, op=ALU.add)
                nc.vector.tensor_reduce(st8[:, 2 * ci + 1:2 * ci + 2],
                                        ssp_t[ci][:],
                                        axis=mybir.AxisListType.X, op=ALU.add)
            gps = psum.tile([P, 2 * NCH], F32, tag="mm", name="gps")
            nc.tensor.matmul(gps[:], lhsT=g_sb[:], rhs=st8[:], start=True,
                             stop=True)
            gnw8 = consts.tile([P, NCH], F32, tag="gnw8")
            gnb8 = consts.tile([P, NCH], F32, tag="gnb8")
            for ci in range(NCH):
                nc.vector.tensor_copy(gnw8[:, ci:ci + 1], bpk_sb[ci][:, 2:3])
                nc.vector.tensor_copy(gnb8[:, ci:ci + 1], bpk_sb[ci][:, 3:4])
            mean = consts.tile([P, NCH], F32, tag="mean")
            nc.vector.tensor_scalar_mul(mean[:], gps[:, 0:2 * NCH:2], CNT_INV)
            ex2 = consts.tile([P, NCH], F32, tag="ex2")
            nc.vector.tensor_scalar_mul(ex2[:], gps[:, 1:2 * NCH:2], CNT_INV)
            msq = consts.tile([P, NCH], F32, tag="msq")
            nc.vector.tensor_mul(msq[:], mean[:], mean[:])
            vpe = consts.tile([P, NCH], F32, tag="vpe")
            # (ex2 + EPS) - mean^2
            nc.vector.scalar_tensor_tensor(vpe[:], in0=ex2[:], scalar=EPS,
                                           in1=msq[:], op0=ALU.add,
                                           op1=ALU.subtract)
            rvar = consts.tile([P, NCH], F32, tag="rvar")
            nc.vector.reciprocal(rvar[:], vpe[:])
            rstd = consts.tile([P, NCH], F32, tag="rstd")
            nc.scalar.activation(rstd[:], rvar[:], AF.Sqrt)
            Aall = consts.tile([P, NCH], F32, tag="Aall")
            nc.vector.tensor_mul(Aall[:], rstd[:], gnw8[:])
            nmA = consts.tile([P, NCH], F32, tag="nmA")
            # (mean * -1) * A
            nc.vector.scalar_tensor_tensor(nmA[:], in0=mean[:], scalar=-1.0,
                                           in1=Aall[:], op0=ALU.mult,
                                           op1=ALU.mult)
            Ball = consts.tile([P, NCH], F32, tag="Ball")
            nc.vector.tensor_add(Ball[:], nmA[:], gnb8[:])
            A_t = [Aall[:, ci:ci + 1] for ci in range(NCH)]
            B_t = [Ball[:, ci:ci + 1] for ci in range(NCH)]

            # ---- h = A*x + B, column-block-major, split across DVE/ACT ----
            for jb in range(NSEQ // 512):
                for ci in range(NCH):
                    sl = slice(jb * 512, (jb + 1) * 512)
                    if (jb * NCH + ci) % 2 == 0:
                        nc.vector.tensor_scalar(h_sb[ci][:, sl],
                                                xs_t[ci][:, sl],
                                                A_t[ci], B_t[ci],
                                                op0=ALU.mult, op1=ALU.add)
                    else:
                        nc.scalar.activation(h_sb[ci][:, sl], xs_t[ci][:, sl],
                                             AF.Identity, bias=B_t[ci],
                                             scale=A_t[ci])

        # close the streaming pool; attention-phase pools may now reuse its
        # SBUF range
        xsp_cm.__exit__(None, None, None)
        kqp = ctx.enter_context(tc.tile_pool(name="kqp", bufs=1))
        attp = ctx.enter_context(tc.tile_pool(name="attp", bufs=1))
        outp = ctx.enter_context(tc.tile_pool(name="outp", bufs=1))

        # ---- projections ----
        # vT[j] : 32 tiles of [128 (j), 512 (c)], bias row via K=1 matmul
        vt_sb = []
        for jt in range(NJ):
            ps = psum.tile([P, C], F32, tag="mm", name=f"vps{jt}")
            for ci in range(NCH):
                nc.tensor.matmul(ps[:], lhsT=h_sb[ci][:, jt * P:(jt + 1) * P],
                                 rhs=wt["v"][ci][:], start=(ci == 0),
                                 stop=(not with_vbias and ci == NCH - 1))
            if with_vbias:
                nc.tensor.matmul(ps[:], lhsT=ones_row[:], rhs=bvr_sb[:],
                                 start=False, stop=True)
            vtt = kqp.tile([P, C], BF16, tag="vt", bufs=NJ, name=f"vt{jt}")
            nc.vector.tensor_copy(vtt[:], ps[:])
            vt_sb.append(vtt)
        # k[co, j] : 4 chunks of [128, 4096]
        k_sb = []
        for co in range(NCH):
            t = kqp.tile([P, NSEQ], BF16, tag=f"k{co}", name=f"k{co}")
            k_sb.append(t)
        for co in range(NCH):
            for jt in range(NSEQ // 512):
                ps = psum.tile([P, 512], F32, tag="mm", name=f"kps{co}_{jt}")
                for ci in range(NCH):
                    nc.tensor.matmul(ps[:], lhsT=wt["k"][ci][:, co * P:(co + 1) * P],
                                     rhs=h_sb[ci][:, jt * 512:(jt + 1) * 512],
                                     start=(ci == 0), stop=(ci == NCH - 1))
                nc.scalar.activation(k_sb[co][:, jt * 512:(jt + 1) * 512], ps[:],
                                     AF.Identity, bias=bpk_sb[co][:, 1:2])
        # q[co, i] : 4 chunks of [128, 2048] (own query half = cols 0:2048)
        q_sb = []
        for co in range(NCH):
            t = kqp.tile([P, NQ], BF16, tag=f"q{co}", name=f"q{co}")
            q_sb.append(t)
        for it in range(NQ // 512):
            for co in range(NCH):
                ps = psum.tile([P, 512], F32, tag="mm", name=f"qps{co}_{it}")
                for ci in range(NCH):
                    nc.tensor.matmul(ps[:], lhsT=wt["q"][ci][:, co * P:(co + 1) * P],
                                     rhs=h_sb[ci][:, it * 512:(it + 1) * 512],
                                     start=(ci == 0), stop=(ci == NCH - 1))
                nc.scalar.activation(q_sb[co][:, it * 512:(it + 1) * 512], ps[:],
                                     AF.Identity, bias=bpk_sb[co][:, 0:1])

        # ---- attention + fused output projection ----
        for ic in range(NI):
            accs = [psum.tile([P, 512], F32, tag="acc", name=f"acc{ic}_{c}")
                    for c in range(NCH)]
            eacc_prev = None
            for jt in range(NJ):
                ps = psum.tile([P, 512], F32, tag="mm", name=f"sps{ic}_{jt}")
                for ci in range(NCH):
                    nc.tensor.matmul(ps[:], lhsT=k_sb[ci][:, jt * P:(jt + 1) * P],
                                     rhs=q_sb[ci][:, ic * 512:(ic + 1) * 512],
                                     start=(ci == 0), stop=(ci == NCH - 1))
                et = attp.tile([P, 512], BF16, tag="et", bufs=3,
                               name=f"et{ic}_{jt}")
                nc.scalar.activation(et[:], ps[:], AF.Exp, scale=SCALE)
                eacc = attp.tile([P, 512], F32, tag="ea", bufs=2,
                                 name=f"ea{ic}_{jt}")
                if jt == 0:
                    nc.vector.tensor_copy(eacc[:], et[:])
                else:
                    nc.vector.tensor_add(eacc[:], eacc_prev[:], et[:])
                eacc_prev = eacc
                for c in range(NCH):
                    nc.tensor.matmul(accs[c][:], lhsT=vt_sb[jt][:, c * P:(c + 1) * P],
                                     rhs=et[:], start=(jt == 0), stop=(jt == NJ - 1))
            h2c = []
            for c in range(NCH):
                h2t = attp.tile([P, 512], BF16, tag="h2", bufs=2 * NCH,
                                name=f"h2_{ic}_{c}")
                nc.scalar.copy(h2t[:], accs[c][:])
                h2c.append(h2t)
            rcs = []
            gate_inst = None
            for iq in range(4):
                dps = psum.tile([P, 1], F32, tag="mm", name=f"dps{ic}_{iq}")
                mm_i = nc.tensor.matmul(dps[:],
                                        lhsT=eacc_prev[:, iq * P:(iq + 1) * P],
                                        rhs=ones_col[:], start=True, stop=True)
                if iq == 0:
                    gate_inst = mm_i
                rc = consts.tile([P, 1], F32, tag=f"rc{ic * 4 + iq}",
                                 name=f"rc{ic * 4 + iq}")
                nc.vector.reciprocal(rc[:], dps[:])
                rcs.append(rc)
            # output projection for this i-chunk (transposed) + residual
            for iq in range(4):
                t_i = ic * 4 + iq
                pps = psum.tile([P, C], F32, tag="mm", name=f"pps{t_i}")
                for c in range(NCH):
                    nc.tensor.matmul(pps[:], lhsT=h2c[c][:, iq * P:(iq + 1) * P],
                                     rhs=wt["p"][c][:], start=(c == 0),
                                     stop=(c == NCH - 1))
                xt = outp.tile([P, C], F32, tag="xr", bufs=3, name=f"xt{t_i}")
                xt_dma = nc.sync.dma_start(xt[:], xpbT_d[t_i * P:(t_i + 1) * P, :])
                # keep the residual loads out of the phase-A DMA window: only
                # issue them once this i-chunk's attention is winding down
                add_dep_helper(xt_dma.ins, gate_inst.ins, sync=True,
                               reason="delay residual load")
                ot = outp.tile([P, C], F32, tag="ot", bufs=3, name=f"ot{t_i}")
                nc.vector.scalar_tensor_tensor(ot[:], in0=pps[:],
                                               scalar=rcs[iq][:], in1=xt[:],
                                               op0=ALU.mult, op1=ALU.add)
                nc.sync.dma_start(out_d[t_i * P:(t_i + 1) * P, :], ot[:])

    nc.compile()
    if not nc.is_finalized():
        nc.finalize()
    return nc


_NC_CACHE = {}


def _get_nc(with_vbias=True):
    if with_vbias not in _NC_CACHE:
        _NC_CACHE[with_vbias] = build_nc(with_vbias)
    return _NC_CACHE[with_vbias]


def make_in_maps(x, gn_w, gn_b, wq, bq, wk, bk, wv, bv, wp, bp):
    bf = ml_dtypes.bfloat16
    x = np.asarray(x, np.float32)
    B = x.shape[0]
    shared = {
        "wqT": np.ascontiguousarray(np.asarray(wq, np.float32).T).astype(bf),
        "wkT": np.ascontiguousarray(np.asarray(wk, np.float32).T).astype(bf),
        "wvT": np.ascontiguousarray(np.asarray(wv, np.float32).T).astype(bf),
        "wpT": np.ascontiguousarray(np.asarray(wp, np.float32).T).astype(bf),
        "bpk": np.ascontiguousarray(
            np.stack([bq, bk, gn_w, gn_b], axis=1).astype(np.float32)),
        "bvr": np.asarray(bv, np.float32).reshape(1, C).astype(bf),
        "gmat": np.kron(np.eye(8, dtype=np.float32),
                        np.ones((16, 16), np.float32)),
    }
    in_maps = []
    for core in range(2 * B):
        b, h = divmod(core, 2)
        xb2 = x[b].reshape(C, NSEQ)
        own = xb2[:, h * NQ:(h + 1) * NQ]
        other = xb2[:, (1 - h) * NQ:(2 - h) * NQ]
        m = dict(shared)
        m["x"] = np.ascontiguousarray(
            np.concatenate([own, other], axis=1)).astype(bf)
        m["xpbT"] = np.ascontiguousarray(own.T + np.asarray(bp, np.float32)[None, :])
        in_maps.append(m)
    return in_maps


def kernel(x, gn_w, gn_b, wq, bq, wk, bk, wv, bv, wp, bp, _run_kwargs=None):
    x = np.asarray(x)
    B, C_, H, W = x.shape
    with_vbias = bool(np.any(np.asarray(bv, np.float32)))
    nc = _get_nc(with_vbias)
    in_maps = make_in_maps(x, gn_w, gn_b, wq, bq, wk, bk, wv, bv, wp, bp)
    res = run_bass_kernel_spmd(nc, in_maps, list(range(2 * B)),
                               **(_run_kwargs or {}))
    out = np.empty((B, C, NSEQ), np.float32)
    for core in range(2 * B):
        b, h = divmod(core, 2)
        out[b][:, h * NQ:(h + 1) * NQ] = res.results[core]["outT"].T
    out = out.reshape(B, C, H, W).astype(x.dtype, copy=False)
    kernel.last_results = res
    return out



# revision 4
# speedup vs baseline: 1.7960x; 1.7960x over previous
"""AttnBlock (GroupNorm + 1x1-conv QKV + NxN attention + proj + residual) on 8 NeuronCores.

Sharding: data-parallel over batch (4 samples) x 2-way sequence-parallel over
query rows. Each core gets one sample's full (C,N) activation with its query
half permuted to columns 0:2048, computes GroupNorm stats, normalizes, runs
scores/softmax/AV in a j-transposed layout (so no on-chip transposes are
needed anywhere), and emits its 2048 output columns transposed (positions on
partitions) so the softmax denominator can be applied as a per-partition
scalar.

All heavy matmuls run in fp8(e4m3) with DoubleRow perf mode (two K=128
sub-tiles contracted per instruction) and fp32 PSUM accumulation. Weights are
pre-scaled by 16 on the host so their values sit in fp8's normal range; the
scale factors cancel through the softmax normalization (folded into the exp
scale and the denominator reciprocal). Statistics, softmax denominators and
the residual path stay in fp32.
"""

import numpy as np
import ml_dtypes
from contextlib import ExitStack

import concourse.bass as bass
import concourse.bacc as bacc
import concourse.mybir as mybir
import concourse.tile as tile
from concourse.tile_rust import add_dep_helper
from concourse.bass_utils import run_bass_kernel_spmd

F32 = mybir.dt.float32
BF16 = mybir.dt.bfloat16
F8 = mybir.dt.float8e4
AF = mybir.ActivationFunctionType
ALU = mybir.AluOpType
DR = mybir.MatmulPerfMode.DoubleRow

C = 512          # channels
NSEQ = 4096      # sequence length (H*W)
NQ = 2048        # query rows per core (sequence-parallel 2-way)
P = 128          # partitions
NCH = C // P     # 4 channel chunks
NCP = NCH // 2   # 2 channel chunk pairs (DoubleRow)
NJ = NSEQ // P   # 32 key-position chunks
NJP = NJ // 2    # 16 key-position chunk pairs
NI = NQ // 512   # 4 query chunks of 512
EPS = 1e-6
SCALE = float(C) ** -0.5
CNT_INV = 1.0 / (16 * NSEQ)   # elements per group (16 ch x 4096 positions)

WS = 16.0            # host-side weight scale (keeps fp8 weights normal-range)
SCALE2 = SCALE / (WS * WS)   # exp scale: undoes q*k weight scaling
EXP_OFF = 2.0        # constant subtracted inside exp; cancels in softmax
H2S = 1.0 / 1024.0   # AV-psum -> fp8 rescale
# ot = pps * (1/dps) + xt must equal wp@h2/denom + x + bp.
# pps = (WS*WS*H2S) * (wp @ h2u);  dps = ONEVAL * denom
# => ONEVAL = WS*WS*H2S = 0.25
ONEVAL = WS * WS * H2S


def build_nc(with_vbias=True):
    nc = bacc.Bacc("TRN2", target_bir_lowering=False, debug=False)

    x_d = nc.dram_tensor("x", [C, NSEQ], BF16, kind="ExternalInput")
    wqT_d = nc.dram_tensor("wqT", [C, C], F8, kind="ExternalInput")
    wkT_d = nc.dram_tensor("wkT", [C, C], F8, kind="ExternalInput")
    wvT_d = nc.dram_tensor("wvT", [C, C], F8, kind="ExternalInput")
    wpT_d = nc.dram_tensor("wpT", [C, C], F8, kind="ExternalInput")
    # packed per-channel vectors: cols 0=16*bq 1=16*bk 2=gn_w 3=gn_b
    bpk_d = nc.dram_tensor("bpk", [C, 4], F32, kind="ExternalInput")
    bvr_d = nc.dram_tensor("bvr", [1, C], F8, kind="ExternalInput")
    g_d = nc.dram_tensor("gmat", [P, P], F32, kind="ExternalInput")
    xpbT_d = nc.dram_tensor("xpbT", [NQ, C], F32, kind="ExternalInput")
    out_d = nc.dram_tensor("outT", [NQ, C], F32, kind="ExternalOutput")

    x_3d = x_d.rearrange("(c p) n -> p c n", p=P)
    bpk_3d = bpk_d.rearrange("(c p) k -> p c k", p=P)

    with tile.TileContext(nc) as tc, ExitStack() as ctx:
        psum = ctx.enter_context(tc.tile_pool(name="psum", bufs=4, space="PSUM"))
        consts = ctx.enter_context(tc.tile_pool(name="consts", bufs=1))
        wpool = ctx.enter_context(tc.tile_pool(name="wpool", bufs=1))
        hp = ctx.enter_context(tc.tile_pool(name="hp", bufs=1))
        h4 = hp.tile([P, NCH, NSEQ], F8, tag="h4", name="h4")

        # ---- x loads first: one DMA per channel chunk so the stats can
        # pipeline behind the transfers ----
        xsp_cm = tc.tile_pool(name="xsp", bufs=1)
        xsp = xsp_cm.__enter__()
        xs_t = []
        HS = NSEQ // 2
        for ci in range(NCH):
            xs = xsp.tile([P, NSEQ], BF16, tag=f"xs{ci}", bufs=1,
                          name=f"xs{ci}")
            for hf in range(2):
                nc.sync.dma_start(xs[:, hf * HS:(hf + 1) * HS],
                                  x_3d[:, ci, hf * HS:(hf + 1) * HS])
            xs_t.append(xs)

        # constants (issued after x so their queue slots don't delay it)
        g_sb = consts.tile([P, P], F32, tag="g")
        nc.sync.dma_start(g_sb[:], g_d[:])
        bpk_all = consts.tile([P, NCH, 4], F32, tag="bpk")
        nc.sync.dma_start(bpk_all[:], bpk_3d)
        bpk_sb = [bpk_all[:, ci, :] for ci in range(NCH)]
        bvr_sb = consts.tile([1, C], F8, tag="bvr")
        nc.sync.dma_start(bvr_sb[:], bvr_d[:])
        ones_row = consts.tile([1, P], F8, tag="ones1")
        nc.vector.memset(ones_row[:], 1.0)
        ones_col = consts.tile([P, 1], F32, tag="ones2")
        nc.vector.memset(ones_col[:], ONEVAL)
        noff = consts.tile([P, 1], F32, tag="noff")
        nc.vector.memset(noff[:], -EXP_OFF)

        wt = {}
        for wn, wd in (("q", wqT_d), ("k", wkT_d), ("v", wvT_d), ("p", wpT_d)):
            wall = wpool.tile([P, NCH, C], F8, tag=f"w{wn}", name=f"w{wn}")
            nc.sync.dma_start(wall[:], wd.rearrange("(c p) n -> p c n", p=P))
            wt[wn] = wall

        # ---- per-chunk stats ----
        sp_t, ssp_t = [], []
        for ci in range(NCH):
            xs = xs_t[ci]
            st = consts.tile([P, 2], F32, tag=f"s{ci}", name=f"s{ci}")
            sst = consts.tile([P, 2], F32, tag=f"ss{ci}", name=f"ss{ci}")
            for hf in range(2):
                hsl = slice(hf * HS, (hf + 1) * HS)
                nc.vector.tensor_reduce(st[:, hf:hf + 1], xs[:, hsl],
                                        axis=mybir.AxisListType.X,
                                        op=ALU.add)
                sq = xsp.tile([P, HS], BF16, tag="sq", bufs=2,
                              name=f"sq{ci}_{hf}")
                nc.scalar.activation(sq[:], xs[:, hsl], AF.Square,
                                     accum_out=sst[:, hf:hf + 1])
            sp_t.append(st)
            ssp_t.append(sst)

        # ---- group stats -> per-channel affine A, B (batched; st8
        # columns are (s0,ss0,s1,ss1,...)) ----
        st8 = consts.tile([P, 2 * NCH], F32, tag="st8")
        for ci in range(NCH):
            nc.vector.tensor_reduce(st8[:, 2 * ci:2 * ci + 1], sp_t[ci][:],
                                    axis=mybir.AxisListType.X, op=ALU.add)
            nc.vector.tensor_reduce(st8[:, 2 * ci + 1:2 * ci + 2],
                                    ssp_t[ci][:],
                                    axis=mybir.AxisListType.X, op=ALU.add)
        gps = psum.tile([P, 2 * NCH], F32, tag="mm", name="gps")
        nc.tensor.matmul(gps[:], lhsT=g_sb[:], rhs=st8[:], start=True,
                         stop=True)
        gnw8 = consts.tile([P, NCH], F32, tag="gnw8")
        gnb8 = consts.tile([P, NCH], F32, tag="gnb8")
        for ci in range(NCH):
            nc.vector.tensor_copy(gnw8[:, ci:ci + 1], bpk_sb[ci][:, 2:3])
            nc.vector.tensor_copy(gnb8[:, ci:ci + 1], bpk_sb[ci][:, 3:4])
        mean = consts.tile([P, NCH], F32, tag="mean")
        nc.vector.tensor_scalar_mul(mean[:], gps[:, 0:2 * NCH:2], CNT_INV)
        ex2 = consts.tile([P, NCH], F32, tag="ex2")
        nc.vector.tensor_scalar_mul(ex2[:], gps[:, 1:2 * NCH:2], CNT_INV)
        msq = consts.tile([P, NCH], F32, tag="msq")
        nc.vector.tensor_mul(msq[:], mean[:], mean[:])
        vpe = consts.tile([P, NCH], F32, tag="vpe")
        # (ex2 + EPS) - mean^2
        nc.vector.scalar_tensor_tensor(vpe[:], in0=ex2[:], scalar=EPS,
                                       in1=msq[:], op0=ALU.add,
                                       op1=ALU.subtract)
        rvar = consts.tile([P, NCH], F32, tag="rvar")
        nc.vector.reciprocal(rvar[:], vpe[:])
        rstd = consts.tile([P, NCH], F32, tag="rstd")
        nc.scalar.activation(rstd[:], rvar[:], AF.Sqrt)
        Aall = consts.tile([P, NCH], F32, tag="Aall")
        nc.vector.tensor_mul(Aall[:], rstd[:], gnw8[:])
        nmA = consts.tile([P, NCH], F32, tag="nmA")
        # (mean * -1) * A
        nc.vector.scalar_tensor_tensor(nmA[:], in0=mean[:], scalar=-1.0,
                                       in1=Aall[:], op0=ALU.mult,
                                       op1=ALU.mult)
        Ball = consts.tile([P, NCH], F32, tag="Ball")
        nc.vector.tensor_add(Ball[:], nmA[:], gnb8[:])
        A_t = [Aall[:, ci:ci + 1] for ci in range(NCH)]
        B_t = [Ball[:, ci:ci + 1] for ci in range(NCH)]

        # ---- h = A*x + B (fp8 out), column-block-major, split DVE/ACT ----
        for jb in range(NSEQ // 512):
            for ci in range(NCH):
                sl = slice(jb * 512, (jb + 1) * 512)
                if (jb * NCH + ci) % 2 == 0:
                    nc.vector.tensor_scalar(h4[:, ci, sl],
                                            xs_t[ci][:, sl],
                                            A_t[ci], B_t[ci],
                                            op0=ALU.mult, op1=ALU.add)
                else:
                    nc.scalar.activation(h4[:, ci, sl], xs_t[ci][:, sl],
                                         AF.Identity, bias=B_t[ci],
                                         scale=A_t[ci])

        # close the streaming pool; attention-phase pools may now reuse its
        # SBUF range
        xsp_cm.__exit__(None, None, None)
        kqp = ctx.enter_context(tc.tile_pool(name="kqp", bufs=1))
        attp = ctx.enter_context(tc.tile_pool(name="attp", bufs=1))
        outp = ctx.enter_context(tc.tile_pool(name="outp", bufs=1))

        # ---- projections (all DoubleRow fp8) ----
        # vT pairs: 16 tiles of [128 (j), 2 (j-sub), 512 (c)]
        vt2 = []
        for t in range(NJP):
            vtt = kqp.tile([P, 2, C], F8, tag="vt", bufs=NJP, name=f"vt{t}")
            vt2.append(vtt)
        for jt in range(NJ):
            ps = psum.tile([P, C], F32, tag="mm", name=f"vps{jt}")
            for cp in range(NCP):
                nc.tensor.matmul(ps[:],
                                 lhsT=h4[:, 2 * cp:2 * cp + 2, jt * P:(jt + 1) * P],
                                 rhs=wt["v"][:, 2 * cp:2 * cp + 2, :],
                                 start=(cp == 0),
                                 stop=(not with_vbias and cp == NCP - 1),
                                 perf_mode=DR)
            if with_vbias:
                nc.tensor.matmul(ps[:], lhsT=ones_row[:], rhs=bvr_sb[:],
                                 start=False, stop=True)
            nc.vector.tensor_copy(vt2[jt // 2][:, jt % 2, :], ps[:])
        # k[co, j] : [128, 4, 4096] fp8
        k4 = kqp.tile([P, NCH, NSEQ], F8, tag="k4", name="k4")
        for co in range(NCH):
            for jb in range(NSEQ // 512):
                ps = psum.tile([P, 512], F32, tag="mm", name=f"kps{co}_{jb}")
                for cp in range(NCP):
                    nc.tensor.matmul(ps[:],
                                     lhsT=wt["k"][:, 2 * cp:2 * cp + 2, co * P:(co + 1) * P],
                                     rhs=h4[:, 2 * cp:2 * cp + 2, jb * 512:(jb + 1) * 512],
                                     start=(cp == 0), stop=(cp == NCP - 1),
                                     perf_mode=DR)
                nc.scalar.activation(k4[:, co, jb * 512:(jb + 1) * 512], ps[:],
                                     AF.Identity, bias=bpk_sb[co][:, 1:2])
        # q[co, i] : [128, 4, 2048] fp8 (own query half = cols 0:2048)
        q4 = kqp.tile([P, NCH, NQ], F8, tag="q4", name="q4")
        for it in range(NQ // 512):
            for co in range(NCH):
                ps = psum.tile([P, 512], F32, tag="mm", name=f"qps{co}_{it}")
                for cp in range(NCP):
                    nc.tensor.matmul(ps[:],
                                     lhsT=wt["q"][:, 2 * cp:2 * cp + 2, co * P:(co + 1) * P],
                                     rhs=h4[:, 2 * cp:2 * cp + 2, it * 512:(it + 1) * 512],
                                     start=(cp == 0), stop=(cp == NCP - 1),
                                     perf_mode=DR)
                nc.scalar.activation(q4[:, co, it * 512:(it + 1) * 512], ps[:],
                                     AF.Identity, bias=bpk_sb[co][:, 0:1])

        # ---- attention + fused output projection ----
        for ic in range(NI):
            accs = [psum.tile([P, 512], F32, tag="acc", name=f"acc{ic}_{c}")
                    for c in range(NCH)]
            eacc_prev = None
            av_pending = None   # AV runs one pair-step behind scores so the
            # PE never waits on the exp latency
            for tp in range(NJP):
                sps = []
                for hf in range(2):
                    ps = psum.tile([P, 512], F32, tag="mm",
                                   name=f"sps{ic}_{tp}_{hf}")
                    for cp in range(NCP):
                        nc.tensor.matmul(
                            ps[:],
                            lhsT=k4[:, 2 * cp:2 * cp + 2,
                                    (2 * tp + hf) * P:(2 * tp + hf + 1) * P],
                            rhs=q4[:, 2 * cp:2 * cp + 2,
                                   ic * 512:(ic + 1) * 512],
                            start=(cp == 0), stop=(cp == NCP - 1),
                            perf_mode=DR)
                    sps.append(ps)
                et2 = attp.tile([P, 2, 512], F8, tag="et", bufs=3,
                                name=f"et{ic}_{tp}")
                for hf in range(2):
                    nc.scalar.activation(et2[:, hf, :], sps[hf][:], AF.Exp,
                                         scale=SCALE2, bias=noff[:])
                if tp == 0:
                    ea = attp.tile([P, 512], F32, tag="ea", bufs=2,
                                   name=f"ea{ic}_{tp}")
                    nc.vector.tensor_add(ea[:], et2[:, 0, :], et2[:, 1, :])
                else:
                    ea1 = attp.tile([P, 512], F32, tag="ea", bufs=2,
                                    name=f"ea1_{ic}_{tp}")
                    nc.vector.tensor_add(ea1[:], eacc_prev[:], et2[:, 0, :])
                    ea = attp.tile([P, 512], F32, tag="ea", bufs=2,
                                   name=f"ea{ic}_{tp}")
                    nc.vector.tensor_add(ea[:], ea1[:], et2[:, 1, :])
                eacc_prev = ea
                if av_pending is not None:
                    pet, ptp = av_pending
                    for c in range(NCH):
                        nc.tensor.matmul(accs[c][:],
                                         lhsT=vt2[ptp][:, :, c * P:(c + 1) * P],
                                         rhs=pet[:, :, :],
                                         start=(ptp == 0), stop=False,
                                         perf_mode=DR)
                av_pending = (et2, tp)
            pet, ptp = av_pending
            for c in range(NCH):
                nc.tensor.matmul(accs[c][:],
                                 lhsT=vt2[ptp][:, :, c * P:(c + 1) * P],
                                 rhs=pet[:, :, :],
                                 start=False, stop=True, perf_mode=DR)

            # h2 pairs (rescaled into fp8 range), split ACT/DVE
            h2p = [attp.tile([P, 2, 512], F8, tag=f"h2_{pr}", bufs=2,
                             name=f"h2_{ic}_{pr}") for pr in range(2)]
            for c in range(NCH):
                dst = h2p[c // 2][:, c % 2, :]
                if c % 2 == 0:
                    nc.scalar.activation(dst, accs[c][:], AF.Identity,
                                         scale=H2S)
                else:
                    nc.vector.tensor_scalar_mul(dst, accs[c][:], H2S)
            rcs = []
            gate_inst = None
            for iq in range(4):
                dps = psum.tile([P, 1], F32, tag="mm", name=f"dps{ic}_{iq}")
                mm_i = nc.tensor.matmul(dps[:],
                                        lhsT=eacc_prev[:, iq * P:(iq + 1) * P],
                                        rhs=ones_col[:], start=True, stop=True)
                if iq == 0:
                    gate_inst = mm_i
                rc = consts.tile([P, 1], F32, tag=f"rc{ic * 4 + iq}",
                                 name=f"rc{ic * 4 + iq}")
                nc.vector.reciprocal(rc[:], dps[:])
                rcs.append(rc)
            # output projection for this i-chunk (transposed) + residual
            for iq in range(4):
                t_i = ic * 4 + iq
                pps = psum.tile([P, C], F32, tag="mm", name=f"pps{t_i}")
                for pr in range(2):
                    nc.tensor.matmul(pps[:],
                                     lhsT=h2p[pr][:, :, iq * P:(iq + 1) * P],
                                     rhs=wt["p"][:, 2 * pr:2 * pr + 2, :],
                                     start=(pr == 0), stop=(pr == 1),
                                     perf_mode=DR)
                xt = outp.tile([P, C], F32, tag="xr", bufs=3, name=f"xt{t_i}")
                xt_dma = nc.sync.dma_start(xt[:], xpbT_d[t_i * P:(t_i + 1) * P, :])
                # keep the residual loads out of the phase-A DMA window: only
                # issue them once this i-chunk's attention is winding down
                add_dep_helper(xt_dma.ins, gate_inst.ins, sync=True,
                               reason="delay residual load")
                ot = outp.tile([P, C], F32, tag="ot", bufs=3, name=f"ot{t_i}")
                nc.vector.scalar_tensor_tensor(ot[:], in0=pps[:],
                                               scalar=rcs[iq][:], in1=xt[:],
                                               op0=ALU.mult, op1=ALU.add)
                nc.sync.dma_start(out_d[t_i * P:(t_i + 1) * P, :], ot[:])

    nc.compile()
    if not nc.is_finalized():
        nc.finalize()
    return nc


_NC_CACHE = {}


def _get_nc(with_vbias=True):
    if with_vbias not in _NC_CACHE:
        _NC_CACHE[with_vbias] = build_nc(with_vbias)
    return _NC_CACHE[with_vbias]


def _to_f8(a):
    return np.clip(np.asarray(a, np.float32), -240.0, 240.0).astype(
        ml_dtypes.float8_e4m3)


def make_in_maps(x, gn_w, gn_b, wq, bq, wk, bk, wv, bv, wp, bp):
    x = np.asarray(x, np.float32)
    B = x.shape[0]
    bf = ml_dtypes.bfloat16
    shared = {
        "wqT": _to_f8(np.asarray(wq, np.float32).T * WS),
        "wkT": _to_f8(np.asarray(wk, np.float32).T * WS),
        "wvT": _to_f8(np.asarray(wv, np.float32).T * WS),
        "wpT": _to_f8(np.asarray(wp, np.float32).T * WS),
        "bpk": np.ascontiguousarray(
            np.stack([WS * np.asarray(bq, np.float32),
                      WS * np.asarray(bk, np.float32),
                      np.asarray(gn_w, np.float32),
                      np.asarray(gn_b, np.float32)], axis=1)),
        "bvr": _to_f8(WS * np.asarray(bv, np.float32).reshape(1, C)),
        "gmat": np.kron(np.eye(8, dtype=np.float32),
                        np.ones((16, 16), np.float32)),
    }
    in_maps = []
    for core in range(2 * B):
        b, h = divmod(core, 2)
        xb2 = x[b].reshape(C, NSEQ)
        own = xb2[:, h * NQ:(h + 1) * NQ]
        other = xb2[:, (1 - h) * NQ:(2 - h) * NQ]
        m = dict(shared)
        m["x"] = np.ascontiguousarray(
            np.concatenate([own, other], axis=1)).astype(bf)
        m["xpbT"] = np.ascontiguousarray(own.T + np.asarray(bp, np.float32)[None, :])
        in_maps.append(m)
    return in_maps


def kernel(x, gn_w, gn_b, wq, bq, wk, bk, wv, bv, wp, bp, _run_kwargs=None):
    x = np.asarray(x)
    B, C_, H, W = x.shape
    with_vbias = bool(np.any(np.asarray(bv, np.float32)))
    nc = _get_nc(with_vbias)
    in_maps = make_in_maps(x, gn_w, gn_b, wq, bq, wk, bk, wv, bv, wp, bp)
    res = run_bass_kernel_spmd(nc, in_maps, list(range(2 * B)),
                               **(_run_kwargs or {}))
    out = np.empty((B, C, NSEQ), np.float32)
    for core in range(2 * B):
        b, h = divmod(core, 2)
        out[b][:, h * NQ:(h + 1) * NQ] = res.results[core]["outT"].T
    out = out.reshape(B, C, H, W).astype(x.dtype, copy=False)
    kernel.last_results = res
    return out


# revision 8
# speedup vs baseline: 1.9428x; 1.0817x over previous
"""AttnBlock (GroupNorm + 1x1-conv QKV + NxN attention + proj + residual) on 8 NeuronCores.

Sharding: data-parallel over batch (4 samples) x 2-way sequence-parallel over
query rows. Each core gets one sample's full (C,N) activation with its query
half permuted to columns 0:2048, computes GroupNorm stats, normalizes, runs
scores/softmax/AV in a j-transposed layout (so no on-chip transposes are
needed anywhere), and emits its 2048 output columns transposed (positions on
partitions) so the softmax denominator can be applied as a per-partition
scalar.

All heavy matmuls run in fp8(e4m3) with DoubleRow perf mode (two K=128
sub-tiles contracted per instruction) and fp32 PSUM accumulation. Weights are
pre-scaled by 16 on the host so their values sit in fp8's normal range; the
scale factors cancel through the softmax normalization (folded into the exp
scale and the denominator reciprocal). Statistics, softmax denominators and
the residual path stay in fp32/bf16.

Scheduling: QKV phase is emitted per-512-column block (affine -> v -> k -> q)
so PSUM evacuations never queue behind unrelated engine work; the per-i-chunk
output block (denominators, projection, residual) is deferred into the next
i-chunk's score stream so the PE never drains at chunk boundaries.
"""

import numpy as np
import ml_dtypes
from contextlib import ExitStack

import concourse.bass as bass
import concourse.bacc as bacc
import concourse.mybir as mybir
import concourse.tile as tile
from concourse.tile_rust import add_dep_helper
from concourse.bass_utils import run_bass_kernel_spmd

F32 = mybir.dt.float32
BF16 = mybir.dt.bfloat16
F8 = mybir.dt.float8e4
AF = mybir.ActivationFunctionType
ALU = mybir.AluOpType
DR = mybir.MatmulPerfMode.DoubleRow

C = 512          # channels
NSEQ = 4096      # sequence length (H*W)
NQ = 2048        # query rows per core (sequence-parallel 2-way)
P = 128          # partitions
NCH = C // P     # 4 channel chunks
NCP = NCH // 2   # 2 channel chunk pairs (DoubleRow)
NJ = NSEQ // P   # 32 key-position chunks
NJP = NJ // 2    # 16 key-position chunk pairs
NI = NQ // 512   # 4 query chunks of 512
EPS = 1e-6
SCALE = float(C) ** -0.5
CNT_INV = 1.0 / (16 * NSEQ)   # elements per group (16 ch x 4096 positions)

WS = 16.0            # host-side weight scale (keeps fp8 weights normal-range)
SCALE2 = SCALE / (WS * WS)   # exp scale: undoes q*k weight scaling
EXP_OFF = 2.0        # constant subtracted inside exp; cancels in softmax
H2S = 1.0 / 1024.0   # AV-psum -> fp8 rescale
# ot = pps * (1/dps) + xt must equal wp@h2/denom + x + bp.
# pps = (WS*WS*H2S) * (wp @ h2u);  dps = ONEVAL * denom
# => ONEVAL = WS*WS*H2S = 0.25
ONEVAL = WS * WS * H2S


def build_nc(with_vbias=True):
    nc = bacc.Bacc("TRN2", target_bir_lowering=False, debug=False)

    x_d = nc.dram_tensor("x", [C, NSEQ], BF16, kind="ExternalInput")
    wqT_d = nc.dram_tensor("wqT", [C, C], F8, kind="ExternalInput")
    wkT_d = nc.dram_tensor("wkT", [C, C], F8, kind="ExternalInput")
    wvT_d = nc.dram_tensor("wvT", [C, C], F8, kind="ExternalInput")
    wpT_d = nc.dram_tensor("wpT", [C, C], F8, kind="ExternalInput")
    # packed per-channel vectors: cols 0=16*bq 1=16*bk 2=gn_w 3=gn_b
    bpk_d = nc.dram_tensor("bpk", [C, 4], F32, kind="ExternalInput")
    bvr_d = nc.dram_tensor("bvr", [1, C], F8, kind="ExternalInput")
    g_d = nc.dram_tensor("gmat", [P, P], F32, kind="ExternalInput")
    xpbT_d = nc.dram_tensor("xpbT", [NQ, C], F32, kind="ExternalInput")
    out_d = nc.dram_tensor("outT", [NQ, C], F32, kind="ExternalOutput")

    x_3d = x_d.rearrange("(c p) n -> p c n", p=P)
    bpk_3d = bpk_d.rearrange("(c p) k -> p c k", p=P)

    with tile.TileContext(nc) as tc, ExitStack() as ctx:
        psum = ctx.enter_context(tc.tile_pool(name="psum", bufs=4, space="PSUM"))
        consts = ctx.enter_context(tc.tile_pool(name="consts", bufs=1))
        wpool = ctx.enter_context(tc.tile_pool(name="wpool", bufs=1))
        hp = ctx.enter_context(tc.tile_pool(name="hp", bufs=1))
        h4 = hp.tile([P, NCH, NSEQ], F8, tag="h4", name="h4")

        # ---- x loads first: one DMA per channel chunk so the stats can
        # pipeline behind the transfers ----
        xsp = ctx.enter_context(tc.tile_pool(name="xsp", bufs=1))
        xs_t = []
        HS = NSEQ // 2
        for ci in range(NCH):
            xs = xsp.tile([P, NSEQ], BF16, tag=f"xs{ci}", bufs=1,
                          name=f"xs{ci}")
            for hf in range(2):
                nc.sync.dma_start(xs[:, hf * HS:(hf + 1) * HS],
                                  x_3d[:, ci, hf * HS:(hf + 1) * HS])
            xs_t.append(xs)

        # constants (issued after x so their queue slots don't delay it)
        g_sb = consts.tile([P, P], F32, tag="g")
        nc.sync.dma_start(g_sb[:], g_d[:])
        bpk_all = consts.tile([P, NCH, 4], F32, tag="bpk")
        nc.sync.dma_start(bpk_all[:], bpk_3d)
        bpk_sb = [bpk_all[:, ci, :] for ci in range(NCH)]
        bvr_sb = consts.tile([1, C], F8, tag="bvr")
        nc.sync.dma_start(bvr_sb[:], bvr_d[:])
        ones_row = consts.tile([1, P], F8, tag="ones1")
        nc.vector.memset(ones_row[:], 1.0)
        ones_col = consts.tile([P, 1], BF16, tag="ones2")
        nc.vector.memset(ones_col[:], ONEVAL)
        noff = consts.tile([P, 1], F32, tag="noff")
        nc.vector.memset(noff[:], -EXP_OFF)

        wt = {}
        for wn, wd in (("q", wqT_d), ("k", wkT_d), ("v", wvT_d), ("p", wpT_d)):
            wall = wpool.tile([P, NCH, C], F8, tag=f"w{wn}", name=f"w{wn}")
            nc.sync.dma_start(wall[:], wd.rearrange("(c p) n -> p c n", p=P))
            wt[wn] = wall

        # ---- per-chunk stats: sum on GpSimd (otherwise idle), sum-of-squares
        # via ACT Square+accum.  Keeps the DVE queue clear for the affine and
        # PSUM evacuations that gate the PE. ----
        sp_t, ssp_t = [], []
        for ci in range(NCH):
            xs = xs_t[ci]
            st = consts.tile([P, 2], F32, tag=f"s{ci}", name=f"s{ci}")
            sst = consts.tile([P, 2], F32, tag=f"ss{ci}", name=f"ss{ci}")
            for hf in range(2):
                hsl = slice(hf * HS, (hf + 1) * HS)
                nc.vector.tensor_reduce(st[:, hf:hf + 1], xs[:, hsl],
                                        axis=mybir.AxisListType.X,
                                        op=ALU.add)
                sq = xsp.tile([P, HS], BF16, tag="sq", bufs=2,
                              name=f"sq{ci}_{hf}")
                nc.scalar.activation(sq[:], xs[:, hsl], AF.Square,
                                     accum_out=sst[:, hf:hf + 1])
            sp_t.append(st)
            ssp_t.append(sst)

        # ---- group stats -> per-channel affine A, B (batched; st8
        # columns are (s0,ss0,s1,ss1,...)) ----
        st8 = consts.tile([P, 2 * NCH], F32, tag="st8")
        for ci in range(NCH):
            nc.vector.tensor_reduce(st8[:, 2 * ci:2 * ci + 1], sp_t[ci][:],
                                    axis=mybir.AxisListType.X, op=ALU.add)
            nc.vector.tensor_reduce(st8[:, 2 * ci + 1:2 * ci + 2],
                                    ssp_t[ci][:],
                                    axis=mybir.AxisListType.X, op=ALU.add)
        gps = psum.tile([P, 2 * NCH], F32, tag="mm", name="gps")
        nc.tensor.matmul(gps[:], lhsT=g_sb[:], rhs=st8[:], start=True,
                         stop=True)
        gnw8 = consts.tile([P, NCH], F32, tag="gnw8")
        gnb8 = consts.tile([P, NCH], F32, tag="gnb8")
        for ci in range(NCH):
            nc.vector.tensor_copy(gnw8[:, ci:ci + 1], bpk_sb[ci][:, 2:3])
            nc.vector.tensor_copy(gnb8[:, ci:ci + 1], bpk_sb[ci][:, 3:4])
        mean = consts.tile([P, NCH], F32, tag="mean")
        nc.vector.tensor_scalar_mul(mean[:], gps[:, 0:2 * NCH:2], CNT_INV)
        ex2 = consts.tile([P, NCH], F32, tag="ex2")
        nc.vector.tensor_scalar_mul(ex2[:], gps[:, 1:2 * NCH:2], CNT_INV)
        msq = consts.tile([P, NCH], F32, tag="msq")
        nc.vector.tensor_mul(msq[:], mean[:], mean[:])
        vpe = consts.tile([P, NCH], F32, tag="vpe")
        # (ex2 + EPS) - mean^2
        nc.vector.scalar_tensor_tensor(vpe[:], in0=ex2[:], scalar=EPS,
                                       in1=msq[:], op0=ALU.add,
                                       op1=ALU.subtract)
        rvar = consts.tile([P, NCH], F32, tag="rvar")
        nc.vector.reciprocal(rvar[:], vpe[:])
        rstd = consts.tile([P, NCH], F32, tag="rstd")
        nc.scalar.activation(rstd[:], rvar[:], AF.Sqrt)
        Aall = consts.tile([P, NCH], F32, tag="Aall")
        nc.vector.tensor_mul(Aall[:], rstd[:], gnw8[:])
        nmA = consts.tile([P, NCH], F32, tag="nmA")
        # (mean * -1) * A
        nc.vector.scalar_tensor_tensor(nmA[:], in0=mean[:], scalar=-1.0,
                                       in1=Aall[:], op0=ALU.mult,
                                       op1=ALU.mult)
        Ball = consts.tile([P, NCH], F32, tag="Ball")
        nc.vector.tensor_add(Ball[:], nmA[:], gnb8[:])
        A_t = [Aall[:, ci:ci + 1] for ci in range(NCH)]
        B_t = [Ball[:, ci:ci + 1] for ci in range(NCH)]

        kqp = ctx.enter_context(tc.tile_pool(name="kqp", bufs=1))
        attp = ctx.enter_context(tc.tile_pool(name="attp", bufs=1))
        outp = ctx.enter_context(tc.tile_pool(name="outp", bufs=1))

        vt2 = [kqp.tile([P, 2, C], F8, tag="vt", bufs=NJP, name=f"vt{t}")
               for t in range(NJP)]
        k4 = kqp.tile([P, NCH, NSEQ], F8, tag="k4", name="k4")
        q4 = kqp.tile([P, NCH, NQ], F8, tag="q4", name="q4")

        # ---- fused normalize + QKV, per 512-column block: the affine that
        # produces a block is immediately followed by the v/k/q matmuls that
        # consume it, and each PSUM evacuation is issued right behind its
        # matmul so no engine queue backs up. ----
        for jb in range(NSEQ // 512):
            jsl = slice(jb * 512, (jb + 1) * 512)
            for ci in range(NCH):
                if (jb * NCH + ci) % 2 == 0:
                    nc.vector.tensor_scalar(h4[:, ci, jsl], xs_t[ci][:, jsl],
                                            A_t[ci], B_t[ci],
                                            op0=ALU.mult, op1=ALU.add)
                else:
                    nc.scalar.activation(h4[:, ci, jsl], xs_t[ci][:, jsl],
                                         AF.Identity, bias=B_t[ci],
                                         scale=A_t[ci])
            # v for the 4 j-chunks of this block (evacuate on DVE)
            for t in range(4):
                jt = 4 * jb + t
                ps = psum.tile([P, C], F32, tag="mm", name=f"vps{jt}")
                for cp in range(NCP):
                    nc.tensor.matmul(ps[:],
                                     lhsT=h4[:, 2 * cp:2 * cp + 2,
                                             jt * P:(jt + 1) * P],
                                     rhs=wt["v"][:, 2 * cp:2 * cp + 2, :],
                                     start=(cp == 0),
                                     stop=(not with_vbias and cp == NCP - 1),
                                     perf_mode=DR)
                if with_vbias:
                    nc.tensor.matmul(ps[:], lhsT=ones_row[:], rhs=bvr_sb[:],
                                     start=False, stop=True)
                nc.vector.tensor_copy(vt2[jt // 2][:, jt % 2, :], ps[:])
            # k for all 4 output-channel chunks at this block (evacuate on ACT)
            for co in range(NCH):
                ps = psum.tile([P, 512], F32, tag="mm", name=f"kps{co}_{jb}")
                for cp in range(NCP):
                    nc.tensor.matmul(ps[:],
                                     lhsT=wt["k"][:, 2 * cp:2 * cp + 2,
                                                  co * P:(co + 1) * P],
                                     rhs=h4[:, 2 * cp:2 * cp + 2, jsl],
                                     start=(cp == 0), stop=(cp == NCP - 1),
                                     perf_mode=DR)
                nc.scalar.activation(k4[:, co, jsl], ps[:],
                                     AF.Identity, bias=bpk_sb[co][:, 1:2])
            # q (first half of the columns only; evacuate on DVE)
            if jb < NQ // 512:
                for co in range(NCH):
                    ps = psum.tile([P, 512], F32, tag="mm",
                                   name=f"qps{co}_{jb}")
                    for cp in range(NCP):
                        nc.tensor.matmul(ps[:],
                                         lhsT=wt["q"][:, 2 * cp:2 * cp + 2,
                                                      co * P:(co + 1) * P],
                                         rhs=h4[:, 2 * cp:2 * cp + 2, jsl],
                                         start=(cp == 0), stop=(cp == NCP - 1),
                                         perf_mode=DR)
                    nc.vector.tensor_scalar(q4[:, co, jsl], ps[:],
                                            bpk_sb[co][:, 0:1], None,
                                            op0=ALU.add)

        # ---- attention + fused output projection ----
        # Output work for i-chunk `ic` (denominators, projection, residual,
        # store) is emitted two pair-steps into i-chunk `ic+1`, so the PE
        # stays on the score/AV stream across the boundary.
        pending = None

        def emit_output_block(blk):
            ic, h2p, eacc = blk
            rcs = []
            gate_inst = None
            for iq in range(4):
                dps = psum.tile([P, 1], F32, tag="mm", name=f"dps{ic}_{iq}")
                mm_i = nc.tensor.matmul(dps[:],
                                        lhsT=eacc[:, iq * P:(iq + 1) * P],
                                        rhs=ones_col[:], start=True, stop=True)
                if iq == 0:
                    gate_inst = mm_i
                rc = consts.tile([P, 1], F32, tag=f"rc{ic * 4 + iq}",
                                 name=f"rc{ic * 4 + iq}")
                nc.vector.reciprocal(rc[:], dps[:])
                rcs.append(rc)
            for iq in range(4):
                t_i = ic * 4 + iq
                pps = psum.tile([P, C], F32, tag="mm", name=f"pps{t_i}")
                for pr in range(2):
                    nc.tensor.matmul(pps[:],
                                     lhsT=h2p[pr][:, :, iq * P:(iq + 1) * P],
                                     rhs=wt["p"][:, 2 * pr:2 * pr + 2, :],
                                     start=(pr == 0), stop=(pr == 1),
                                     perf_mode=DR)
                xt = outp.tile([P, C], F32, tag="xr", bufs=3, name=f"xt{t_i}")
                xt_dma = nc.sync.dma_start(xt[:],
                                           xpbT_d[t_i * P:(t_i + 1) * P, :])
                # keep the residual loads out of the phase-A DMA window
                add_dep_helper(xt_dma.ins, gate_inst.ins, sync=True,
                               reason="delay residual load")
                ot = outp.tile([P, C], F32, tag="ot", bufs=3, name=f"ot{t_i}")
                nc.vector.scalar_tensor_tensor(ot[:], in0=pps[:],
                                               scalar=rcs[iq][:], in1=xt[:],
                                               op0=ALU.mult, op1=ALU.add)
                nc.sync.dma_start(out_d[t_i * P:(t_i + 1) * P, :], ot[:])

        for ic in range(NI):
            accs = [psum.tile([P, 512], F32, tag="acc", name=f"acc{ic}_{c}")
                    for c in range(NCH)]
            eacc_prev = None
            av_pending = None   # AV runs one pair-step behind scores so the
            # PE never waits on the exp latency
            for tp in range(NJP):
                sps = []
                for hf in range(2):
                    ps = psum.tile([P, 512], F32, tag="mm",
                                   name=f"sps{ic}_{tp}_{hf}")
                    for cp in range(NCP):
                        nc.tensor.matmul(
                            ps[:],
                            lhsT=k4[:, 2 * cp:2 * cp + 2,
                                    (2 * tp + hf) * P:(2 * tp + hf + 1) * P],
                            rhs=q4[:, 2 * cp:2 * cp + 2,
                                   ic * 512:(ic + 1) * 512],
                            start=(cp == 0), stop=(cp == NCP - 1),
                            perf_mode=DR)
                    sps.append(ps)
                et2 = attp.tile([P, 2, 512], F8, tag="et", bufs=3,
                                name=f"et{ic}_{tp}")
                for hf in range(2):
                    nc.scalar.activation(et2[:, hf, :], sps[hf][:], AF.Exp,
                                         scale=SCALE2, bias=noff[:])
                if tp == 0:
                    ea = attp.tile([P, 512], BF16, tag="ea", bufs=2,
                                   name=f"ea{ic}_{tp}")
                    nc.vector.tensor_add(ea[:], et2[:, 0, :], et2[:, 1, :])
                else:
                    ea1 = attp.tile([P, 512], BF16, tag="ea", bufs=2,
                                    name=f"ea1_{ic}_{tp}")
                    nc.vector.tensor_add(ea1[:], eacc_prev[:], et2[:, 0, :])
                    ea = attp.tile([P, 512], BF16, tag="ea", bufs=2,
                                   name=f"ea{ic}_{tp}")
                    nc.vector.tensor_add(ea[:], ea1[:], et2[:, 1, :])
                eacc_prev = ea
                if av_pending is not None:
                    pet, ptp = av_pending
                    for c in range(NCH):
                        nc.tensor.matmul(accs[c][:],
                                         lhsT=vt2[ptp][:, :, c * P:(c + 1) * P],
                                         rhs=pet[:, :, :],
                                         start=(ptp == 0), stop=False,
                                         perf_mode=DR)
                av_pending = (et2, tp)
                if pending is not None and tp == 2:
                    emit_output_block(pending)
                    pending = None
            pet, ptp = av_pending
            for c in range(NCH):
                nc.tensor.matmul(accs[c][:],
                                 lhsT=vt2[ptp][:, :, c * P:(c + 1) * P],
                                 rhs=pet[:, :, :],
                                 start=False, stop=True, perf_mode=DR)

            # h2 pairs (rescaled into fp8 range), split ACT/DVE; these free
            # the acc PSUM banks for the next i-chunk
            h2p = [attp.tile([P, 2, 512], F8, tag=f"h2_{pr}", bufs=2,
                             name=f"h2_{ic}_{pr}") for pr in range(2)]
            for c in range(NCH):
                dst = h2p[c // 2][:, c % 2, :]
                if c % 2 == 0:
                    nc.scalar.activation(dst, accs[c][:], AF.Identity,
                                         scale=H2S)
                else:
                    nc.vector.tensor_scalar_mul(dst, accs[c][:], H2S)
            pending = (ic, h2p, eacc_prev)
        emit_output_block(pending)

    nc.compile()
    if not nc.is_finalized():
        nc.finalize()
    return nc


_NC_CACHE = {}


def _get_nc(with_vbias=True):
    if with_vbias not in _NC_CACHE:
        _NC_CACHE[with_vbias] = build_nc(with_vbias)
    return _NC_CACHE[with_vbias]


def _to_f8(a):
    return np.clip(np.asarray(a, np.float32), -240.0, 240.0).astype(
        ml_dtypes.float8_e4m3)


def make_in_maps(x, gn_w, gn_b, wq, bq, wk, bk, wv, bv, wp, bp):
    x = np.asarray(x, np.float32)
    B = x.shape[0]
    bf = ml_dtypes.bfloat16
    shared = {
        "wqT": _to_f8(np.asarray(wq, np.float32).T * WS),
        "wkT": _to_f8(np.asarray(wk, np.float32).T * WS),
        "wvT": _to_f8(np.asarray(wv, np.float32).T * WS),
        "wpT": _to_f8(np.asarray(wp, np.float32).T * WS),
        "bpk": np.ascontiguousarray(
            np.stack([WS * np.asarray(bq, np.float32),
                      WS * np.asarray(bk, np.float32),
                      np.asarray(gn_w, np.float32),
                      np.asarray(gn_b, np.float32)], axis=1)),
        "bvr": _to_f8(WS * np.asarray(bv, np.float32).reshape(1, C)),
        "gmat": np.kron(np.eye(8, dtype=np.float32),
                        np.ones((16, 16), np.float32)),
    }
    in_maps = []
    for core in range(2 * B):
        b, h = divmod(core, 2)
        xb2 = x[b].reshape(C, NSEQ)
        own = xb2[:, h * NQ:(h + 1) * NQ]
        other = xb2[:, (1 - h) * NQ:(2 - h) * NQ]
        m = dict(shared)
        m["x"] = np.ascontiguousarray(
            np.concatenate([own, other], axis=1)).astype(bf)
        m["xpbT"] = np.ascontiguousarray(own.T + np.asarray(bp, np.float32)[None, :])
        in_maps.append(m)
    return in_maps


def kernel(x, gn_w, gn_b, wq, bq, wk, bk, wv, bv, wp, bp, _run_kwargs=None):
    x = np.asarray(x)
    B, C_, H, W = x.shape
    with_vbias = bool(np.any(np.asarray(bv, np.float32)))
    nc = _get_nc(with_vbias)
    in_maps = make_in_maps(x, gn_w, gn_b, wq, bq, wk, bk, wv, bv, wp, bp)
    res = run_bass_kernel_spmd(nc, in_maps, list(range(2 * B)),
                               **(_run_kwargs or {}))
    out = np.empty((B, C, NSEQ), np.float32)
    for core in range(2 * B):
        b, h = divmod(core, 2)
        out[b][:, h * NQ:(h + 1) * NQ] = res.results[core]["outT"].T
    out = out.reshape(B, C, H, W).astype(x.dtype, copy=False)
    kernel.last_results = res
    return out
